# revision 2
# baseline (speedup 1.0000x reference)
"""Kernel builder for CrossLayerAttention on one NeuronCore (2 examples).

Math (per example, per-pixel over an 80x80 query grid; key/value grid 40x40):
  q = wq @ query + bq; k = wk @ key + bk; v = wv @ key + bv
  ki = bilinear_up2(k); vi = bilinear_up2(v)   (align_corners=False)
  score[h] = SCALE * sum_{c in head h} q[c]*ki[c]
  attn = softmax over x (height) of score
  out = wo @ (4 * attn_bcast * vi) + bo

Key implementation tricks:
  - fp32r matmuls for f32 data (full PE rate at N>=256); bf16 matmuls for the
    bf16 score/attn path.
  - SCALE folded into the score mask matmul; bilinear weights folded into k/v
    evictions (sigma=0.5625); R2=4 folded into v's eviction scale.
  - Upsample intermediates in bf16 with a padded + shifted-copy layout so every
    scalar_tensor_tensor hits the DVE 2x_1P packed mode (4B alignment).
  - y (width) is stored parity-split (y = par*40 + l) downstream of the
    horizontal upsample so its outputs are step-1; the final eviction
    un-permutes while copying PSUM->SBUF.
  - Evictions use two-bank PSUM tiles (FD 640 per instruction) to amortize the
    per-op SBUF latency bubble.
"""
from contextlib import ExitStack

import numpy as np
from concourse.bass_utils import run_bass_kernel_spmd

import concourse.bass as bass
import concourse.bacc as bacc
import concourse.tile as tile
from concourse import mybir
from concourse.masks import make_identity

F32 = mybir.dt.float32
F32R = mybir.dt.float32r
BF16 = mybir.dt.bfloat16
AOP = mybir.AluOpType

B_LOC = 2
C = 256
H = W = 80
KH = KW = 40
NPIX = H * W            # 6400
KPIX = KH * KW          # 1600
HEADS = 4
SCALE = 1.0 / 16.0
SIG_K = 0.5625          # 0.75^2
SIG_V = 4.0 * 0.5625    # R2 folded in
KNT = 400               # k/v projection pixel tile (one psum bank <= 512)
XS = 16                 # upsample strip height (output x rows); 5 strips
NSTRIP = H // XS
NT = 640                # stage C/E double tile (8 x-rows, 2 psum banks)
NMM = 320               # matmul N (one psum bank)
DBL = XS * W // NT      # double tiles per strip (2)
PW = 46                 # padded kv75 width: [1]=left pad, [2..41]=data, [42]=right pad


class Cfg:
    up_dt = BF16
    ki_dt = BF16
    vi_dt = BF16
    e_dt = BF16
    attn_eng = "gpsimd"  # E*R multiply
    prod_dt = BF16


def build(cfg=Cfg):
    nc = bacc.Bacc(target_bir_lowering=False, trn_type="TRN2")
    query_d = nc.dram_tensor("query", [B_LOC, C, NPIX], BF16, kind="ExternalInput")
    key_d = nc.dram_tensor("key", [B_LOC, C, KPIX], BF16, kind="ExternalInput")
    w_d = {n: nc.dram_tensor(n, [C, C], F32, kind="ExternalInput")
           for n in ("wq", "wk", "wv", "wo")}
    b_d = {n: nc.dram_tensor(n, [C], F32, kind="ExternalInput")
           for n in ("bq", "bk", "bv", "bo")}
    out_d = nc.dram_tensor("out", [B_LOC, C, NPIX], BF16, kind="ExternalOutput")

    with tile.TileContext(nc) as tc:
        with ExitStack() as ctx:
            _body(ctx, tc, nc, cfg, query_d, key_d, w_d, b_d, out_d)
    nc.compile()
    return nc


def _body(ctx, tc, nc, cfg, query_d, key_d, w_d, b_d, out_d):
    const = ctx.enter_context(tc.tile_pool(name="const", bufs=1))
    wtmp = ctx.enter_context(tc.tile_pool(name="wtmp", bufs=2))
    psum_w = ctx.enter_context(tc.tile_pool(name="psum_w", bufs=3, space="PSUM"))
    ident = const.tile([128, 128], F32)
    make_identity(nc, ident)

    # --- weights: load [o, c]; PE-transpose to wT[cj] = [c_part, o_free] ---
    wT = {}

    def prep_weight(n):
        wdt = BF16
        wT[n] = [const.tile([128, 256], wdt, name=f"{n}T{j}") for j in range(2)]
        for oi in range(2):
            wraw = wtmp.tile([128, 256], F32, tag="wraw", name="wraw")
            nc.sync.dma_start(out=wraw, in_=w_d[n][oi * 128:(oi + 1) * 128, :])
            for cj in range(2):
                pt = psum_w.tile([128, 2, 512], F32, tag="ring", name="pt")
                nc.tensor.transpose(pt[:, 0, 0:128], wraw[:, cj * 128:(cj + 1) * 128], ident)
                nc.scalar.copy(out=wT[n][cj][:, oi * 128:(oi + 1) * 128], in_=pt[:, 0, 0:128])

    prep_weight("wk")
    prep_weight("wv")

    # --- biases as [128, 2] (partition = c % 128, free = c // 128) ---
    bt = {}
    for n in ("bq", "bk", "bv", "bo"):
        bt[n] = const.tile([128, 2], F32, name=f"{n}_t")
        nc.sync.dma_start(out=bt[n], in_=b_d[n].ap().rearrange("(j c) -> c j", c=128))
    # pre-scaled biases for the k/v evictions (ACT computes f(in*scale + bias))
    bts = {}
    for n, sc in (("bk", SIG_K), ("bv", SIG_V)):
        bts[n] = const.tile([128, 2], F32, name=f"{n}_ts")
        nc.vector.tensor_scalar(out=bts[n], in0=bt[n], scalar1=sc, scalar2=None,
                                op0=AOP.mult)

    # --- masks (inline constants): score mask [128, 4] w/ SCALE; expand [4, 128] ---
    mdt = BF16 if cfg.prod_dt == BF16 else F32R
    mask_s = [const.tile([128, HEADS], mdt, name=f"mask_s{j}") for j in range(2)]
    mask_e = [const.tile([HEADS, 128], mdt, name=f"mask_e{j}") for j in range(2)]
    for j in range(2):
        ms = np.zeros((128, HEADS), np.float32)
        me = np.zeros((HEADS, 128), np.float32)
        for hh in range(2):
            ms[hh * 64:(hh + 1) * 64, j * 2 + hh] = SCALE
            me[j * 2 + hh, hh * 64:(hh + 1) * 64] = 1.0
        ms_d = nc.inline_tensor(ms, name=f"ms_d{j}")
        me_d = nc.inline_tensor(me, name=f"me_d{j}")
        nc.gpsimd.dma_start(out=mask_s[j], in_=ms_d.ap())
        nc.gpsimd.dma_start(out=mask_e[j], in_=me_d.ap())

    # --- pools ---
    kv_in = ctx.enter_context(tc.tile_pool(name="kv_in", bufs=2))
    kvt = ctx.enter_context(tc.tile_pool(name="kvt", bufs=1))
    kv75 = ctx.enter_context(tc.tile_pool(name="kv75", bufs=1))
    ki_pool = ctx.enter_context(tc.tile_pool(name="ki_pool", bufs=2))
    vi_pool = ctx.enter_context(tc.tile_pool(name="vi_pool", bufs=3))
    qc_pool = ctx.enter_context(tc.tile_pool(name="qc_pool", bufs=3))
    e_pool = ctx.enter_context(tc.tile_pool(name="e_pool", bufs=2))
    sm_pool = ctx.enter_context(tc.tile_pool(name="sm_pool", bufs=1))
    oc_pool = ctx.enter_context(tc.tile_pool(name="oc_pool", bufs=2))
    psum_s = ctx.enter_context(tc.tile_pool(name="psum_s", bufs=1, space="PSUM"))

    # per-example state dicts
    ST = [dict() for _ in range(B_LOC)]

    def stage_a(e):
        # ================= stage A: k, v projections =================
        kt = kvt.tile([128, 2, KH, KW], cfg.up_dt, tag="kt", name="kt")
        vt = kvt.tile([128, 2, KH, KW], cfg.up_dt, tag="vt", name="vt")
        ST[e]["kt"], ST[e]["vt"] = kt, vt
        kt_f = kt.rearrange("p c2 a b -> p c2 (a b)")
        vt_f = vt.rearrange("p c2 a b -> p c2 (a b)")
        for t in range(KPIX // (2 * KNT)):
            sl = slice(t * 2 * KNT, (t + 1) * 2 * KNT)
            key_t = kv_in.tile([128, 2, 2 * KNT], BF16, tag="key_t", name="key_t")
            for kc in range(2):
                nc.sync.dma_start(out=key_t[:, kc, :],
                                  in_=key_d[e, kc * 128:(kc + 1) * 128, sl])
            for m in range(2):
                pk = psum_w.tile([128, 2, 512], F32, tag="ring", name="pk")
                for half in range(2):
                    for kc in range(2):
                        nc.tensor.matmul(
                            pk[:, half, 0:KNT], wT["wk"][kc][:, m * 128:(m + 1) * 128],
                            key_t[:, kc, half * KNT:(half + 1) * KNT],
                            start=(kc == 0), stop=(kc == 1))
                # ACT eviction: out = (psum + bk) * sig  ==  Identity(psum*sig + bk*sig)
                nc.scalar.activation(
                    out=kt_f[:, m, sl].rearrange("p (h n) -> p h n", h=2),
                    in_=pk[:, :, 0:KNT],
                    func=mybir.ActivationFunctionType.Identity,
                    bias=bts["bk"][:, m:m + 1], scale=SIG_K)
                pv = psum_w.tile([128, 2, 512], F32, tag="ring", name="pv")
                for half in range(2):
                    for kc in range(2):
                        nc.tensor.matmul(
                            pv[:, half, 0:KNT], wT["wv"][kc][:, m * 128:(m + 1) * 128],
                            key_t[:, kc, half * KNT:(half + 1) * KNT],
                            start=(kc == 0), stop=(kc == 1))
                nc.scalar.activation(
                    out=vt_f[:, m, sl].rearrange("p (h n) -> p h n", h=2),
                    in_=pv[:, :, 0:KNT],
                    func=mybir.ActivationFunctionType.Identity,
                    bias=bts["bv"][:, m:m + 1], scale=SIG_V)

    # ============== stages B/C/E fused over strips ==============
    XS2 = 40  # strip height (output x rows); divides H; XS2*W % NT == 0
    NHALF = XS2 * W // NT  # double-tiles per strip

    def get_et(e):
        if "e_t" not in ST[e]:
            ST[e]["e_t"] = e_pool.tile([HEADS, NPIX], cfg.e_dt, tag="e_t", name="e_t")
        return ST[e]["e_t"]

    def upsample_strip(src_t, dst, s, which):
        """Bilinear-2x src_t [128,2,40,40] (pre-scaled by sigma) rows
        x' in [s*XS2, (s+1)*XS2) into dst [128, 2, XS2, 2, 40] (parity-split y).

        vertical (scalar_tensor_tensor, 1x): kvA = 0.75 * vertical-bilinear
        kvB = kvA / 3 (tensor_scalar, 4x bf16)
        horizontal (tensor_tensor, 2x bf16):
          ki[2l]   = kvB[l-1] + kvA[l]  (l=1..39); ki[0]  = kvA[0]+kvB[0]
          ki[2l+1] = kvA[l] + kvB[l+1]  (l=0..38); ki[79] = kvA[39]+kvB[39]
        """
        x0 = s * XS2
        j0, jn = x0 // 2, XS2 // 2
        je, jen = (j0, jn) if j0 >= 1 else (1, jn - 1)
        jon = jn if j0 + jn - 1 <= KH - 2 else jn - 1
        kvA = kv75.tile([128, 2, XS2, KW], cfg.up_dt, tag=f"kvA_{which}",
                        name=f"kvA_{which}")
        kvB = kv75.tile([128, 2, XS2, KW], cfg.up_dt, tag=f"kvB_{which}",
                        name=f"kvB_{which}")
        for c2 in range(2):
            nc.vector.scalar_tensor_tensor(
                out=kvA[:, c2, (2 * je - x0):(2 * (je + jen - 1) - x0) + 1:2, :],
                in0=src_t[:, c2, je - 1:je - 1 + jen, :],
                scalar=1.0 / 3.0, in1=src_t[:, c2, je:je + jen, :],
                op0=AOP.mult, op1=AOP.add)
            nc.vector.scalar_tensor_tensor(
                out=kvA[:, c2, (2 * j0 + 1 - x0):(2 * (j0 + jon - 1) + 1 - x0) + 1:2, :],
                in0=src_t[:, c2, j0 + 1:j0 + 1 + jon, :],
                scalar=1.0 / 3.0, in1=src_t[:, c2, j0:j0 + jon, :],
                op0=AOP.mult, op1=AOP.add)
            if j0 == 0:
                nc.vector.tensor_scalar(
                    out=kvA[:, c2, 0, :], in0=src_t[:, c2, 0, :],
                    scalar1=4.0 / 3.0, scalar2=None, op0=AOP.mult)
            if j0 + jn - 1 == KH - 1:
                nc.vector.tensor_scalar(
                    out=kvA[:, c2, XS2 - 1, :], in0=src_t[:, c2, KH - 1, :],
                    scalar1=4.0 / 3.0, scalar2=None, op0=AOP.mult)
        nc.vector.tensor_scalar(
            out=kvB.rearrange("p c2 a b -> p (c2 a b)"),
            in0=kvA.rearrange("p c2 a b -> p (c2 a b)"),
            scalar1=1.0 / 3.0, scalar2=None, op0=AOP.mult)
        for c2 in range(2):
            nc.vector.tensor_tensor(
                out=dst[:, c2, :, 0, 1:40], in0=kvB[:, c2, :, 0:39],
                in1=kvA[:, c2, :, 1:40], op=AOP.add)
            nc.vector.tensor_tensor(
                out=dst[:, c2, :, 1, 0:39], in0=kvA[:, c2, :, 0:39],
                in1=kvB[:, c2, :, 1:40], op=AOP.add)
            nc.vector.tensor_tensor(
                out=dst[:, c2, :, 0, 0:1], in0=kvA[:, c2, :, 0:1],
                in1=kvB[:, c2, :, 0:1], op=AOP.add)
            nc.vector.tensor_tensor(
                out=dst[:, c2, :, 1, 39:40], in0=kvA[:, c2, :, 39:40],
                in1=kvB[:, c2, :, 39:40], op=AOP.add)

    def strip(e, s):
        """Upsample strip s of k and v, then stage C (q/prod/score/exp) on it."""
        e_t = get_et(e)
        ki_t = ki_pool.tile([128, 2, XS2, 2, KW], cfg.ki_dt, tag="ki_t",
                            name="ki_t")
        vi_h = vi_pool.tile([128, 2, XS2, 2, KW], cfg.vi_dt, tag="vi_h",
                            name="vi_h")
        ST[e].setdefault("vi_halves", {})[s] = vi_h
        upsample_strip(ST[e]["kt"], ki_t, s, "k")
        upsample_strip(ST[e]["vt"], vi_h, s, "v")

        ki_f = ki_t.rearrange("p c2 a q b -> p c2 (a q b)")
        for dd_s in range(NHALF):
            dd = s * NHALF + dd_s
            p0 = dd * NT
            q_in = kv_in.tile([128, 2, NT], BF16, tag="q_in", name="q_in")
            for kc in range(2):
                nc.sync.dma_start(
                    out=q_in[:, kc, :],
                    in_=query_d[e, kc * 128:(kc + 1) * 128, p0:p0 + NT])
            prod_t = qc_pool.tile([128, 2, NT], cfg.prod_dt, tag="prod_t",
                                  name="prod_t")
            for m in range(2):
                pq = psum_w.tile([128, 2, 512], F32, tag="ring", name="pq")
                for half in range(2):
                    for kc in range(2):
                        nc.tensor.matmul(
                            pq[:, half, 0:NMM],
                            wT["wq"][kc][:, m * 128:(m + 1) * 128],
                            q_in[:, kc, half * NMM:(half + 1) * NMM],
                            start=(kc == 0), stop=(kc == 1))
                qe = qc_pool.tile([128, NT], cfg.prod_dt, tag="qe", name="qe")
                # write qe permuted (y = 2l+par stored at par*40+l) to match ki
                qe_v = qe.rearrange("p (x y) -> p x y", y=W)
                for half in range(2):
                    qh = qe_v[:, half * 4:(half + 1) * 4, :]
                    qe_perm = bass.AP(
                        tensor=qh.tensor, offset=qh.offset,
                        ap=[list(qh.ap[0]), list(qh.ap[1]), [1, KW], [KW, 2]])
                    nc.scalar.add(out=qe_perm,
                                  in_=pq[:, half, 0:NMM].rearrange(
                                      "p (x y) -> p x y", y=W),
                                  add=bt["bq"][:, m:m + 1])
                prod_eng = nc.vector if dd % 2 == 0 else nc.gpsimd
                prod_eng.tensor_tensor(
                    out=prod_t[:, m, :], in0=qe,
                    in1=ki_f[:, m, dd_s * NT:(dd_s + 1) * NT], op=AOP.mult)
            ps = psum_s.tile([HEADS, 2, 512], F32, tag="psa", name="ps")
            for half in range(2):
                for kc in range(2):
                    nc.tensor.matmul(
                        ps[:, half, 0:NMM], mask_s[kc],
                        prod_t[:, kc, half * NMM:(half + 1) * NMM],
                        start=(kc == 0), stop=(kc == 1))
            nc.scalar.activation(
                out=e_t[:, p0:p0 + NT].rearrange("p (h n) -> p h n", h=2),
                in_=ps[:, :, 0:NMM],
                func=mybir.ActivationFunctionType.Exp)

    def stage_d(e):
        # tree-fold x-reduction: level 1 DVE (bf16 2x), middles GPSIMD,
        # final strided 5-way reduce on DVE; then reciprocal.
        e_t = ST[e]["e_t"]
        e_x = e_t.rearrange("p (x y) -> p x y", x=H)
        f1 = sm_pool.tile([HEADS, 40, W], cfg.e_dt, tag="f1", name="f1")
        nc.vector.tensor_tensor(out=f1, in0=e_x[:, 0:40, :], in1=e_x[:, 40:80, :],
                                op=AOP.add)
        f2 = sm_pool.tile([HEADS, 20, W], cfg.e_dt, tag="f2", name="f2")
        nc.vector.tensor_tensor(out=f2, in0=f1[:, 0:20, :], in1=f1[:, 20:40, :],
                                op=AOP.add)
        f3 = sm_pool.tile([HEADS, 10, W], cfg.e_dt, tag="f3", name="f3")
        nc.vector.tensor_tensor(out=f3, in0=f2[:, 0:10, :], in1=f2[:, 10:20, :],
                                op=AOP.add)
        f4 = sm_pool.tile([HEADS, 5, W], F32, tag="f4", name="f4")
        nc.vector.tensor_tensor(out=f4, in0=f3[:, 0:5, :], in1=f3[:, 5:10, :],
                                op=AOP.add)
        s_sum = sm_pool.tile([HEADS, W], F32, tag="s_sum", name="s_sum")
        nc.vector.tensor_reduce(out=s_sum, in_=f4.rearrange("p a b -> p b a"),
                                axis=mybir.AxisListType.X, op=AOP.add)
        r_t = sm_pool.tile([HEADS, W], F32, tag="r_t", name="r_t")
        nc.vector.reciprocal(out=r_t, in_=s_sum)
        r_b16 = sm_pool.tile([HEADS, W], cfg.e_dt, tag="r_b16", name="r_b16", bufs=2)
        nc.vector.tensor_copy(out=r_b16, in_=r_t)
        ST[e]["r_b16"] = r_b16

    attn_eng = {"gpsimd": nc.gpsimd, "vector": nc.vector}[cfg.attn_eng]

    def stage_e(e, tiles):
        e_t = ST[e]["e_t"]
        r_b16 = ST[e]["r_b16"]
        for t in tiles:
            p0 = t * NT
            half = t // NHALF
            off = p0 - half * XS2 * W
            vi_f = ST[e]["vi_halves"][half].rearrange("p c2 a q b -> p c2 (a q b)")
            attn_t = oc_pool.tile([HEADS, NT], cfg.prod_dt, tag="attn_t",
                                  name="attn_t", bufs=4)
            e_view = e_t[:, p0:p0 + NT].rearrange("p (x y) -> p x y", y=W)
            r_ap = r_b16[:, :]
            r_bc = bass.AP(tensor=r_ap.tensor, offset=r_ap.offset,
                           ap=[list(r_ap.ap[0]), [0, NT // W], list(r_ap.ap[1])])
            (nc.vector if t % 2 else nc.gpsimd).tensor_tensor(
                out=attn_t.rearrange("p (x y) -> p x y", y=W),
                in0=e_view, in1=r_bc, op=AOP.mult)
            o_t = oc_pool.tile([128, 2, NT], BF16, tag="o_t", name="o_t", bufs=3)
            for m in range(2):
                pa = psum_w.tile([128, 2, 512], F32, tag="ring", name="pa")
                for half2 in range(2):
                    nc.tensor.matmul(pa[:, half2, 0:NMM], mask_e[m],
                                     attn_t[:, half2 * NMM:(half2 + 1) * NMM],
                                     start=True, stop=True)
                nc.vector.tensor_tensor(
                    out=o_t[:, m, :].rearrange("p (h n) -> p h n", h=2),
                    in0=pa[:, :, 0:NMM],
                    in1=vi_f[:, m, off:off + NT].rearrange("p (h n) -> p h n", h=2),
                    op=AOP.mult)
            res_t = oc_pool.tile([128, 2, NT], BF16, tag="res_t", name="res_t")
            for m in range(2):
                po = psum_w.tile([128, 2, 512], F32, tag="ring", name="po")
                for half2 in range(2):
                    for kc in range(2):
                        nc.tensor.matmul(
                            po[:, half2, 0:NMM],
                            wT["wo"][kc][:, m * 128:(m + 1) * 128],
                            o_t[:, kc, half2 * NMM:(half2 + 1) * NMM],
                            start=(kc == 0), stop=(kc == 1))
                # un-permute y while evicting (psum cols (x, par, l))
                res_v = res_t[:, m, :].rearrange("p (x y) -> p x y", y=W)
                for half2 in range(2):
                    rh = res_v[:, half2 * 4:(half2 + 1) * 4, :]
                    res_perm = bass.AP(
                        tensor=rh.tensor, offset=rh.offset,
                        ap=[list(rh.ap[0]), list(rh.ap[1]), [1, 2], [2, KW]])
                    nc.scalar.add(out=res_perm,
                                  in_=po[:, half2, 0:NMM].rearrange(
                                      "p (x q l) -> p x q l", q=2, l=KW),
                                  add=bt["bo"][:, m:m + 1])
                nc.sync.dma_start(out=out_d[e, m * 128:(m + 1) * 128, p0:p0 + NT],
                                  in_=res_t[:, m, :])

    # ---------------- emission: interleave the two examples ----------------
    NS = H // XS2           # strips per example (2)
    NTOT = NPIX // NT       # E tiles per example (10)
    stage_a(0)
    prep_weight("wq")
    prep_weight("wo")
    for s in range(NS):
        strip(0, s)
    stage_a(1)
    stage_d(0)
    strip(1, 0)
    stage_e(0, range(0, NTOT // 2))
    strip(1, 1)
    stage_e(0, range(NTOT // 2, NTOT))
    stage_d(1)
    stage_e(1, range(NTOT))


_NC_CACHE = {}


def _get_nc():
    if "nc" not in _NC_CACHE:
        _NC_CACHE["nc"] = build()
    return _NC_CACHE["nc"]


def kernel(query, key, wq, bq, wk, bk, wv, bv, wo, bo):
    """Full-input entry point: shards the batch over 8 NeuronCores (2 examples
    per core, pure data parallelism), runs the Bass kernel, gathers the output.

    query [16, 256, 80, 80] f32; key [16, 256, 40, 40] f32;
    w* [256, 256]; b* [256]. Returns [16, 256, 80, 80] f32.
    """
    B, Cq, Hq, Wq = query.shape
    n_cores = 8
    b_loc = B // n_cores
    assert b_loc == B_LOC
    import ml_dtypes
    q_flat = np.ascontiguousarray(query.reshape(B, Cq, Hq * Wq)).astype(
        ml_dtypes.bfloat16)
    k_flat = np.ascontiguousarray(
        key.reshape(B, key.shape[1], key.shape[2] * key.shape[3])).astype(
        ml_dtypes.bfloat16)
    in_maps = []
    for c in range(n_cores):
        in_maps.append({
            "query": q_flat[c * b_loc:(c + 1) * b_loc],
            "key": k_flat[c * b_loc:(c + 1) * b_loc],
            "wq": np.asarray(wq, np.float32), "bq": np.asarray(bq, np.float32),
            "wk": np.asarray(wk, np.float32), "bk": np.asarray(bk, np.float32),
            "wv": np.asarray(wv, np.float32), "bv": np.asarray(bv, np.float32),
            "wo": np.asarray(wo, np.float32), "bo": np.asarray(bo, np.float32),
        })
    nc = _get_nc()
    res = run_bass_kernel_spmd(nc, in_maps, core_ids=list(range(n_cores)))
    out = np.concatenate(
        [res.results[c]["out"].astype(np.float32) for c in range(n_cores)],
        axis=0)
    return out.reshape(B, Cq, Hq, Wq)

